# revision 31
# baseline (speedup 1.0000x reference)
"""Trainium2 Bass kernel for causal multi-head attention with RoPE.

Full module: qkv = x @ w_qkv; RoPE(q, k); causal softmax attention;
out = attn_out @ w_out.  x: [2, 2048, 1024], 16 heads x 64 dim.

Sharding: 8 cores = 2 batches x 4 head-groups (4 heads/core).  Each core
computes its batch's q/k/v for its heads, runs attention, and produces a
partial [2048, 1024] output through its slice of w_out.  Host sums the 4
partials per batch.

v2: st-major software pipeline, bf16 operands (host-cast), static PSUM
plan.  PE segment order qkv(0) qkv(1) attn(0) qkv(2) attn(1) qkv(3)
attn(2) attn(3) keeps the tensor engine dense (HAM stays warm) while
RoPE (DVE/Pool/DMA) and softmax exp (ACT) trail one segment behind.
"""

import os
import sys

import numpy as np

for _p in ("/opt/trn_rl_repo", "/root/.axon_site/_ro/trn_rl_repo"):
    if os.path.isdir(_p) and _p not in sys.path:
        sys.path.append(_p)

import concourse.bass as bass
import concourse.mybir as mybir
import concourse.tile as tile
from concourse.masks import make_upper_triangular

F32 = mybir.dt.float32
BF16 = mybir.dt.bfloat16

# Problem constants (hardcoded per contest rules)
B = 2
N = 2048
D = 1024
HEADS = 16
DH = 64
N_CORES = 8
HL = HEADS // (N_CORES // B)  # heads per core = 4

QT = 512
NQT = N // QT        # 4 query tiles
KC = D // 128        # 8 contraction chunks for qkv
CT = (HL * DH) // 128  # 2 column tiles for q/k (2 heads per tile)
NSB = N // 128       # 16 seq blocks
OC = CT              # w_out contraction chunks from this core
NOT_ = D // 512      # output column tiles
KPQ = QT // 128      # key chunks per query tile


def build_attention_nc(qt=QT, lag=2, cap_waits=True):
    n, d, hl, dh = N, D, HL, DH
    nhp = hl // 2
    scale = float(dh) ** -0.5
    nc = bass.Bass("TRN2", target_bir_lowering=False, debug=False)

    xT = nc.dram_tensor("xT", [NQT * d, qt], BF16, kind="ExternalInput").ap()
    wq = nc.dram_tensor("wq", [d, hl * dh], BF16, kind="ExternalInput").ap()
    wk = nc.dram_tensor("wk", [d, hl * dh], BF16, kind="ExternalInput").ap()
    wv = nc.dram_tensor("wv", [d, hl * dh], BF16, kind="ExternalInput").ap()
    wo = nc.dram_tensor("wo", [hl * dh, d], BF16, kind="ExternalInput").ap()
    cosT = nc.dram_tensor("cosT", [128, n], BF16, kind="ExternalInput").ap()
    sinT = nc.dram_tensor("sinT", [128, n], BF16, kind="ExternalInput").ap()
    selc = nc.dram_tensor("selc", [4, 2 * 128], BF16, kind="ExternalInput").ap()
    outp = nc.dram_tensor("out_partial", [n, d], BF16, kind="ExternalOutput").ap()

    with tile.TileContext(nc) as tc:
        with (
            tc.tile_pool(name="pers", bufs=1) as pers,
            tc.tile_pool(name="ps", bufs=1, space="PSUM") as ps,
            tc.tile_pool(name="ropet", bufs=3) as ropet,
            tc.tile_pool(name="expp", bufs=6) as expp,
            tc.tile_pool(name="bcp", bufs=4) as bcp,
            tc.tile_pool(name="fo", bufs=4) as fo,
        ):
            # ---- persistent SBUF ----
            x_sb = [
                pers.tile([128, KC, qt], BF16, tag=f"x{s}", name=f"x{s}")
                for s in range(NQT)
            ]
            wq_sb = pers.tile([128, KC, hl * dh], BF16, tag="wq", name="wq")
            wk_sb = pers.tile([128, KC, hl * dh], BF16, tag="wk", name="wk")
            wv_sb = pers.tile([128, KC, hl * dh], BF16, tag="wv", name="wv")
            wo_sb = pers.tile([128, OC, d], BF16, tag="wo", name="wo")
            cos_sb = pers.tile([128, n], BF16, tag="cos", name="cos")
            sin_sb = pers.tile([128, n], BF16, tag="sin", name="sin")
            qT_sb = [pers.tile([128, n], BF16, tag=f"qT{i}", name=f"qT{i}") for i in range(CT)]
            kT_sb = [pers.tile([128, n], BF16, tag=f"kT{i}", name=f"kT{i}") for i in range(CT)]
            v_sb = [
                pers.tile([128, hl, dh + 1], BF16, tag=f"v{i}", name=f"v{i}")
                for i in range(NSB)
            ]
            outT_sb = [pers.tile([128, n], BF16, tag=f"oT{i}", name=f"oT{i}") for i in range(CT)]
            # packed unnormalized AV: per head pair, heads at partitions
            # [0,64) and [64,128)
            u_sb = [
                [pers.tile([128, qt], F32, tag=f"u{hp}_{t}", name=f"u{hp}_{t}") for t in range(NQT)]
                for hp in range(nhp)
            ]
            tri_sb = pers.tile([128, 128], BF16, tag="tri", name="tri")
            # selector matrices (host-built): sel[hp] maps recip row (2hp+g)
            # onto output partitions [64g, 64g+64) in the broadcast matmul
            selc_sb = pers.tile([4, 2, 128], BF16, tag="selc", name="selc")
            sel_sb = [selc_sb[:, hp, :] for hp in range(2)]

            # ---- setup: triangle mask + v ones columns ----
            make_upper_triangular(nc, tri_sb[:], val=1.0, diag=True)
            for sb in range(NSB):
                nc.vector.memset(v_sb[sb][:, :, dh : dh + 1], 1.0)
            nc.sync.dma_start(selc_sb, selc.rearrange("p (hp m) -> p hp m", hp=2))

            # ---- input DMA (bf16, host pre-cast); earliest-needed first,
            # wq/x0 halved so qkv(0) starts sooner ----
            hd = d // 2
            for half in range(2):
                sl = slice(half * hd, (half + 1) * hd)
                nc.sync.dma_start(
                    wq_sb[:, half * KC // 2 : (half + 1) * KC // 2, :],
                    wq[sl, :].rearrange("(kc p) m -> p kc m", p=128),
                )
                nc.sync.dma_start(
                    x_sb[0][:, half * KC // 2 : (half + 1) * KC // 2, :],
                    xT[sl, :].rearrange("(kc p) m -> p kc m", p=128),
                )
            nc.sync.dma_start(wk_sb, wk.rearrange("(kc p) m -> p kc m", p=128))
            nc.sync.dma_start(cos_sb, cosT)
            nc.sync.dma_start(sin_sb, sinT)
            nc.sync.dma_start(wv_sb, wv.rearrange("(kc p) m -> p kc m", p=128))
            for s in range(1, NQT):
                nc.sync.dma_start(
                    x_sb[s],
                    xT[s * d : (s + 1) * d, :].rearrange("(kc p) m -> p kc m", p=128),
                )
            nc.sync.dma_start(wo_sb, wo.rearrange("(kc p) m -> p kc m", p=128))

            def qkv_groups(st, woven):
                """List of zero-arg emit closures, one per qkv group.
                Woven groups single-buffer through the wb bank (av2 holds the
                attention denominator while they run)."""
                qsl = slice(st * qt, (st + 1) * qt)
                ems = []

                def qk_group(ct, qk, w_sb, dst, tag):
                    pq = ps.tile([128, qt], F32, tag=tag, name=f"pq{st}_{ct}_{qk}")
                    for kc in range(KC):
                        nc.tensor.matmul(
                            pq,
                            w_sb[:, kc, ct * 128 : (ct + 1) * 128],
                            x_sb[st][:, kc, :],
                            start=(kc == 0),
                            stop=(kc == KC - 1),
                        )
                    # RoPE
                    raw = ropet.tile([128, qt], BF16, tag="raw", name="raw")
                    nc.vector.tensor_copy(raw, pq)
                    sh = ropet.tile([128, qt], BF16, tag="sh", name="sh")
                    # rotate_half as a partition swap (sign folded in sinT)
                    for hb in range(2):
                        o = hb * 64
                        nc.sync.dma_start(sh[o : o + 32, :], raw[o + 32 : o + 64, :])
                        nc.sync.dma_start(sh[o + 32 : o + 64, :], raw[o : o + 32, :])
                    a = ropet.tile([128, qt], BF16, tag="a", name="a")
                    nc.vector.tensor_tensor(a, raw, cos_sb[:, qsl], mybir.AluOpType.mult)
                    nc.gpsimd.tensor_tensor(sh, sh, sin_sb[:, qsl], mybir.AluOpType.mult)
                    nc.vector.tensor_tensor(dst[ct][:, qsl], a, sh, mybir.AluOpType.add)

                def v_group(j, tag):
                    sb = st * KPQ + j
                    psv = ps.tile([128, hl * dh], F32, tag=tag, name=f"psv{sb}")
                    for kc in range(KC):
                        nc.tensor.matmul(
                            psv,
                            x_sb[st][:, kc, j * 128 : (j + 1) * 128],
                            wv_sb[:, kc, :],
                            start=(kc == 0),
                            stop=(kc == KC - 1),
                        )
                    nc.vector.tensor_copy(
                        v_sb[sb][:, :, 0:dh], psv.rearrange("p (h e) -> p h e", h=hl)
                    )

                gi = 0
                for ct in range(CT):
                    for qk, w_sb, dst in ((0, wq_sb, qT_sb), (1, wk_sb, kT_sb)):
                        tag = "wb" if (woven or gi % 2 == 0) else "av2"
                        ems.append(
                            lambda ct=ct, qk=qk, w_sb=w_sb, dst=dst, tag=tag: qk_group(
                                ct, qk, w_sb, dst, tag
                            )
                        )
                        gi += 1
                for j in range(KPQ):
                    tag = "wb" if (woven or (gi + j) % 2 == 0) else "av2"
                    ems.append(lambda j=j, tag=tag: v_group(j, tag))
                return ems

            def emit_qkv(st):
                for em in qkv_groups(st, woven=False):
                    em()

            attn_state = {}

            def emit_attn_main(t, fillers=(), start=0, reserve=0):
                fillers = list(fillers)
                spread = fillers[: len(fillers) - reserve]
                reserved = fillers[len(fillers) - reserve :]
                qsl = slice(t * qt, (t + 1) * qt)
                ncc = KPQ * (t + 1)
                # packed AV: per head pair one [128, qt] bank (heads at
                # partitions [64g, 64g+64)); dens in a separate bank at
                # partitions {0,32,64,96}
                pav = [
                    ps.tile([128, qt], F32, tag=f"av{hp}", name=f"pav{t}_{hp}")
                    for hp in range(nhp)
                ]
                pden = ps.tile([128, qt], F32, tag="av2", name=f"pden{t}")
                e_ts = {}

                def emit_scores(c):
                    j = c - KPQ * t
                    lo = max(0, j * 128)
                    for hp in range(nhp):
                        pss = ps.tile([128, 2, qt], F32, tag=f"sc{hp}", name=f"pss{t}_{c}_{hp}")
                        for g in range(2):
                            bp = 64 * g
                            nc.tensor.matmul(
                                pss[:, g, :],
                                kT_sb[hp][bp : bp + dh, c * 128 : (c + 1) * 128],
                                qT_sb[hp][bp : bp + dh, qsl],
                                start=True,
                                stop=True,
                            )
                        e_t = expp.tile([128, 2, qt], BF16, tag="e", name="e")
                        if lo > 0:
                            nc.scalar.activation(
                                e_t[:, :, lo:qt], pss[:, :, lo:qt],
                                mybir.ActivationFunctionType.Exp, scale=scale,
                            )
                        else:
                            nc.scalar.activation(
                                e_t, pss, mybir.ActivationFunctionType.Exp, scale=scale
                            )
                        if j >= 0:
                            nc.gpsimd.tensor_tensor(
                                e_t[:, :, lo : lo + 128],
                                e_t[:, :, lo : lo + 128],
                                tri_sb[:, None, :].to_broadcast([128, 2, 128]),
                                mybir.AluOpType.mult,
                            )
                        e_ts[(c, hp)] = e_t

                def emit_av(c):
                    lo = max(0, (c - KPQ * t) * 128)
                    # head-dim MMs: col-tiled pairs (cols [64g, 64g+64))
                    for hp in range(nhp):
                        e_t = e_ts[(c, hp)]
                        for g in range(2):
                            h = 2 * hp + g
                            nc.tensor.matmul(
                                pav[hp][64 * g : 64 * g + dh, lo:qt],
                                v_sb[c][:, h, 0:dh],
                                e_t[:, g, lo:qt],
                                start=(c == 0),
                                stop=(c == ncc - 1),
                            )
                    # denominator MMs: 4 one-column tiles, distinct col groups
                    for hp in range(nhp):
                        e_t = e_ts.pop((c, hp))
                        for g in range(2):
                            h = 2 * hp + g
                            nc.tensor.matmul(
                                pden[32 * h : 32 * h + 1, lo:qt],
                                v_sb[c][:, h, dh : dh + 1],
                                e_t[:, g, lo:qt],
                                start=(c == 0),
                                stop=(c == ncc - 1),
                                tile_position=(0, 32 * h),
                            )

                done = 0
                nspread = len(spread)
                for c in range(ncc):
                    emit_scores(c)
                    if c >= lag:
                        emit_av(c - lag)
                    # weave filler groups between chunks: the PE keeps dense
                    # work that doesn't depend on ACT's exp stream
                    if nspread and c >= start:
                        want = min(
                            nspread, (c - start + 1) * nspread // max(1, ncc - start)
                        )
                        while done < want:
                            spread[done]()
                            done += 1
                for c in range(max(0, ncc - lag), ncc):
                    emit_av(c)
                while done < nspread:
                    spread[done]()
                    done += 1

                # denominator path first (critical: feeds the broadcast MM);
                # recip = exp(-ln(d)) on ACT — same act table set as the
                # softmax exp, and ACT is idle at this point
                dsb = bcp.tile([97, qt], F32, tag="dsb", name="dsb")
                nc.vector.tensor_copy(dsb, pden[0:97, :])
                rin = bcp.tile([hl, qt], F32, tag="rin", name="rin")
                for h in range(hl):
                    nc.sync.dma_start(rin[h : h + 1, :], dsb[32 * h : 32 * h + 1, :])
                lnr = bcp.tile([hl, qt], F32, tag="lnr", name="lnr")
                nc.scalar.activation(lnr, rin, mybir.ActivationFunctionType.Ln)
                rr = bcp.tile([hl, qt], BF16, tag="rr", name="rr")
                with nc.allow_low_precision(reason="bf16 softmax recip"):
                    nc.scalar.activation(
                        rr, lnr, mybir.ActivationFunctionType.Exp, scale=-1.0
                    )
                attn_state[t] = rr

                # unnormalized AV -> SBUF (packed, one copy per head pair)
                for hp in range(nhp):
                    nc.vector.tensor_copy(u_sb[hp][t], pav[hp])

                # reserved fillers keep the PE busy while the recip chain runs
                for f in reserved:
                    f()

            def pso_closures(t):
                ems = []
                for j in range(KPQ):
                    sb = t * KPQ + j
                    for nt in range(NOT_):
                        def em(sb=sb, nt=nt, j=j):
                            pso = ps.tile(
                                [128, 512], F32, tag=f"sc{(j * NOT_ + nt) % 2}",
                                name=f"pso{sb}_{nt}",
                            )
                            for kc in range(OC):
                                nc.tensor.matmul(
                                    pso,
                                    outT_sb[kc][:, sb * 128 : (sb + 1) * 128],
                                    wo_sb[:, kc, nt * 512 : (nt + 1) * 512],
                                    start=(kc == 0),
                                    stop=(kc == OC - 1),
                                )
                            o_t = fo.tile([128, 512], BF16, tag="ot", name="ot")
                            nc.vector.tensor_copy(o_t, pso)
                            nc.sync.dma_start(
                                outp[sb * 128 : (sb + 1) * 128, nt * 512 : (nt + 1) * 512],
                                o_t,
                            )
                        ems.append(em)
                return ems

            def emit_epi_head(t):
                qsl = slice(t * qt, (t + 1) * qt)
                rr = attn_state.pop(t)
                pbc = [
                    ps.tile([128, qt], F32, tag=f"av{hp}", name=f"pbc{t}_{hp}")
                    for hp in range(nhp)
                ]
                for hp in range(nhp):
                    nc.tensor.matmul(pbc[hp], sel_sb[hp], rr, start=True, stop=True)
                for hp in range(nhp):
                    nc.vector.tensor_tensor(
                        outT_sb[hp][:, qsl],
                        u_sb[hp][t],
                        pbc[hp],
                        mybir.AluOpType.mult,
                    )

            def emit_attn_epilogue(t):
                emit_epi_head(t)
                for em in pso_closures(t):
                    em()

            emit_qkv(0)
            emit_attn_main(0, qkv_groups(1, woven=True), start=1, reserve=2)
            emit_attn_epilogue(0)
            emit_attn_main(1, qkv_groups(2, woven=True), reserve=3)
            emit_epi_head(1)
            emit_attn_main(
                2, qkv_groups(3, woven=True) + pso_closures(1), reserve=2
            )
            emit_epi_head(2)
            emit_attn_main(3, pso_closures(2), start=1, reserve=2)
            emit_epi_head(3)
            for em in pso_closures(3):
                em()

    if cap_waits:
        _cap_matmul_waits(nc)
    return nc


_CAPPED_INSTS = {
    "InstMatmult",
    "InstTensorTensor",
    "InstTensorCopy",
    "InstActivation",
    "InstTensorScalarAffineSelect",
    "InstTensorScalar",
    "InstTensorReduce",
    "InstMemset",
    "InstReciprocal",
    "InstLdweights",
    "InstTensorTensorScan",
    "InstIota",
    "InstDMACopy",
    "InstDrain",
}


def _cap_matmul_waits(nc, max_keep=1):
    """Walrus codegen allows only one sync-wait per compute instruction
    (S3 struct wait slots).  Move excess waits onto NoOps inserted just
    before, on the same engine; engines execute in order so the semantics
    are identical."""
    nop_id = 0
    for f in nc.m.functions:
        for blk in f.blocks:
            insts = blk.instructions
            idx = 0
            while idx < len(insts):
                inst = insts[idx]
                if (
                    type(inst).__name__ in _CAPPED_INSTS
                    and inst.sync_info is not None
                    and len(inst.sync_info.on_wait or []) > max_keep
                ):
                    waits = list(inst.sync_info.on_wait)
                    extra, keep = waits[:-max_keep], waits[-max_keep:]
                    inst.sync_info = mybir.SyncInfo(
                        on_wait=keep, on_update=list(inst.sync_info.on_update or [])
                    )
                    for w in extra:
                        nop = mybir.InstNoOp(name=f"I-mmwait-nop-{nop_id}")
                        nop_id += 1
                        nop.engine = inst.engine
                        nop.sync_info = mybir.SyncInfo(on_wait=[w], on_update=[])
                        insts.insert(idx, nop)
                        idx += 1
                idx += 1


def _rope_tables(n, dh):
    """Host-side RoPE tables in transposed, 2-head-stacked, sign-folded form."""
    inv_freq = 1.0 / (10000.0 ** (np.arange(0, dh, 2, dtype=np.float32) / dh))
    t = np.arange(n, dtype=np.float32)
    freqs = np.outer(t, inv_freq).astype(np.float32)  # [n, dh/2]
    emb = np.concatenate([freqs, freqs], axis=-1)  # [n, dh]
    cos = np.cos(emb).astype(np.float32).T  # [dh, n]
    sin = np.sin(emb).astype(np.float32).T
    sin_signed = sin.copy()
    sin_signed[: dh // 2] *= -1.0
    cosT = np.ascontiguousarray(np.tile(cos, (128 // dh, 1)))
    sinT = np.ascontiguousarray(np.tile(sin_signed, (128 // dh, 1)))
    return cosT, sinT


_NC_CACHE = {}


def kernel(x, w_qkv, w_out):
    return run(x, w_qkv, w_out)[0]


def _bf16(a):
    import ml_dtypes

    return np.asarray(a, dtype=np.float32).astype(ml_dtypes.bfloat16)


def run(x, w_qkv, w_out, trace=False, build_kwargs=None):
    from concourse.bass_utils import run_bass_kernel_spmd

    x = np.asarray(x, dtype=np.float32)
    w_qkv = np.asarray(w_qkv, dtype=np.float32)
    w_out = np.asarray(w_out, dtype=np.float32)

    cosT, sinT = _rope_tables(N, DH)
    # selector for the recip-broadcast matmul: selm[row, hp*128 + col] = 1
    # iff row == 2*hp + col//64
    selm = np.zeros((4, 2 * 128), dtype=np.float32)
    for hp in range(2):
        for g in range(2):
            selm[2 * hp + g, hp * 128 + g * 64 : hp * 128 + (g + 1) * 64] = 1.0
    in_maps = []
    for core in range(N_CORES):
        b = core // (N_CORES // B)
        g = core % (N_CORES // B)
        cs = slice(g * HL * DH, (g + 1) * HL * DH)
        # x[b].T is [d, n]; reblock into NQT contiguous [d, qt] column blocks
        xt = np.ascontiguousarray(x[b].T)  # [d, n]
        xt_blocks = np.concatenate(
            [xt[:, s * QT : (s + 1) * QT] for s in range(NQT)], axis=0
        )  # [NQT*d, qt]
        in_maps.append(
            {
                "xT": _bf16(xt_blocks),
                "wq": _bf16(w_qkv[:, cs]),
                "wk": _bf16(w_qkv[:, D:][:, cs]),
                "wv": _bf16(w_qkv[:, 2 * D :][:, cs]),
                "wo": _bf16(w_out[cs, :]),
                "cosT": _bf16(cosT),
                "sinT": _bf16(sinT),
                "selc": _bf16(selm),
            }
        )

    key = repr(sorted((build_kwargs or {}).items()))
    if key not in _NC_CACHE:
        _NC_CACHE[key] = build_attention_nc(**(build_kwargs or {}))
    nc = _NC_CACHE[key]

    res = run_bass_kernel_spmd(
        nc, in_maps, core_ids=list(range(N_CORES)), trace=trace
    )
    out = np.zeros((B, N, D), dtype=np.float32)
    for core in range(N_CORES):
        out[core // (N_CORES // B)] += np.asarray(
            res.results[core]["out_partial"], dtype=np.float32
        )
    return out, res


if __name__ == "__main__":
    rng = np.random.default_rng(0)
    x = rng.standard_normal((B, N, D), dtype=np.float32)
    w_qkv = rng.standard_normal((D, 3 * D), dtype=np.float32) * D**-0.5
    w_out = rng.standard_normal((D, D), dtype=np.float32) * D**-0.5
    out = kernel(x, w_qkv, w_out)
    print("out", out.shape, out.dtype, float(np.abs(out).max()))
